# revision 1
# baseline (speedup 1.0000x reference)
"""Deformable head attention on 8 Trainium2 NeuronCores (Bass/Tile).

Sharding: core c -> batch b = c//4, heads (2*(c%4), 2*(c%4)+1).
Each core samples/attends its two heads and emits a partial (y @ Wm [+bm])
projection [C, HW]; the host sums the 4 partials per batch (unshard).

Slot space per (head, level, h24): slot = hw = k*4096 + w*32 + h4  (16384)
Gather: ap_gather, 2x2 patches (d=4, bf16), groups = (m_loc, h24).
"""
import sys
import numpy as np
import ml_dtypes

sys.path.insert(0, "/opt/trn_rl_repo")

import concourse.bass as bass
import concourse.bacc as bacc
import concourse.tile as tile
from concourse import mybir
from concourse.bass_utils import run_bass_kernel_spmd
from contextlib import ExitStack

F32 = mybir.dt.float32
BF16 = mybir.dt.bfloat16
I16 = mybir.dt.int16
I32 = mybir.dt.int32
AF = mybir.ActivationFunctionType
ALU = mybir.AluOpType
AX = mybir.AxisListType

M, K, L, C = 8, 4, 2, 128
C_v = C // M
B, H, W = 2, 128, 128
HW = H * W
GRIDS = [(64, 64), (128, 128)]
N_CORES = 8
WIN = 1024
NWIN = HW // WIN     # 16

_CACHED = {}


def _build_program():
    P = 128
    nc = bacc.Bacc("TRN2", target_bir_lowering=False, debug=False,
                   num_devices=N_CORES)
    def I(name, shape, dt):
        return nc.dram_tensor(name, shape, dt, kind="ExternalInput").ap()

    zqT_d  = I("zqT",  [C, HW], F32)
    x0T_d  = I("x0T",  [C, 64 * 64], F32)
    x1T_d  = I("x1T",  [C, HW], F32)
    Wcmb_d = I("Wcmb", [C, 48], F32)
    bcmb_d = I("bcmb", [48, 1], F32)
    Wp2_d  = I("Wp2",  [C, 32], F32)
    bp2_d  = I("bp2",  [32, 1], F32)
    F0_d   = I("F0",   [C, C], F32)
    F1_d   = I("F1",   [C, C], F32)
    phix_d = I("phix", [C, 2 * 1024], F32)   # cols l*1024 + sc
    phiy_d = I("phiy", [C, 2 * 1024], F32)
    bm_d   = I("bmv",  [C, 1], F32)
    out_d = nc.dram_tensor("outp", [C, HW], F32, kind="ExternalOutput").ap()

    An_d   = nc.dram_tensor("An_i",   [16, HW], BF16).ap()
    img0_d = nc.dram_tensor("img0_i", [32, 64 * 64 + 64 + 4], BF16).ap()
    img1_d = nc.dram_tensor("img1_i", [32, HW + 128 + 4], BF16).ap()
    part_d = nc.dram_tensor("part_i", [C, HW], F32).ap()

    Ra = np.zeros((16, 128), np.float32)
    for p in range(128):
        Ra[(p // 64) * 8 + (p % 8), p] = 1.0
    Es = np.zeros((16, 2), np.float32)
    for r in range(16):
        Es[r, r // 8] = 1.0

    with tile.TileContext(nc) as tc, ExitStack() as ctx:
        const = ctx.enter_context(tc.tile_pool(name="const", bufs=1))
        Ra_t = const.tile([16, 128], BF16)
        nc.sync.dma_start(Ra_t[:], nc.inline_tensor(Ra.astype(ml_dtypes.bfloat16), name="Ra").ap())
        Es_t = const.tile([16, 2], F32)
        nc.sync.dma_start(Es_t[:], nc.inline_tensor(Es, name="Es").ap())
        F_t = []
        for l, fd in enumerate([F0_d, F1_d]):
            ft = const.tile([C, C], BF16, tag=f"F{l}")
            with tc.tile_pool(name="ftmp", bufs=1) as fp:
                tmp = fp.tile([C, C], F32)
                nc.sync.dma_start(tmp[:], fd)
                nc.vector.tensor_copy(ft[:], tmp[:])
            F_t.append(ft)
        bm_t = const.tile([C, 1], F32)
        nc.sync.dma_start(bm_t[:], bm_d)

        # long-lived pipeline outputs allocated early (stack discipline)
        pipe_out = ctx.enter_context(tc.tile_pool(name="pipeo", bufs=1))
        idx_t = [pipe_out.tile([128, 1024], I16, tag=f"idx{l}", name=f"idxt{l}") for l in range(2)]
        w4_t = [pipe_out.tile([128, 4096], BF16, tag=f"w4{l}", name=f"w4t{l}") for l in range(2)]

        # ========== phase A: DL = Wcmb.T @ zqT + b (zqT streamed) ==========
        DLstack = ExitStack()
        DLpool = DLstack.enter_context(tc.tile_pool(name="DL", bufs=1))
        DLd_t = DLpool.tile([32, HW], F32, tag="DLd")
        DLastack = ExitStack()
        DLapool = DLastack.enter_context(tc.tile_pool(name="DLa", bufs=1))
        DLa_t = DLapool.tile([16, HW], F32, tag="DLa")
        with tc.tile_pool(name="phA", bufs=3) as pa, \
             tc.tile_pool(name="phAp", bufs=2, space="PSUM") as pap:
            Wcmb_t = pa.tile([C, 48], F32, tag="wc")
            nc.sync.dma_start(Wcmb_t[:], Wcmb_d)
            bcmb_t = pa.tile([48, 1], F32, tag="bc")
            nc.sync.dma_start(bcmb_t[:], bcmb_d)
            for w in range(HW // 512):
                zw = pa.tile([C, 512], F32, tag="zw")
                nc.sync.dma_start(zw[:], zqT_d[:, w*512:(w+1)*512])
                d_ps = pap.tile([32, 512], F32, tag="dps")
                nc.tensor.matmul(d_ps[:], Wcmb_t[:, :32], zw[:],
                                 start=True, stop=True)
                nc.scalar.activation(DLd_t[:, w*512:(w+1)*512], d_ps[:],
                                     AF.Identity, bias=bcmb_t[:32, :])
                a_ps = pap.tile([16, 512], F32, tag="aps")
                nc.tensor.matmul(a_ps[:], Wcmb_t[:, 32:48], zw[:],
                                 start=True, stop=True)
                nc.scalar.activation(DLa_t[:, w*512:(w+1)*512], a_ps[:],
                                     AF.Identity, bias=bcmb_t[32:48, :])

        # ========== phase B: softmax -> An (bf16) -> DRAM ==========
        with tc.tile_pool(name="phB", bufs=1) as pb, \
             tc.tile_pool(name="phBp", bufs=2, space="PSUM") as pbp:
            for hq4 in range(4):
                QN = HW // 4
                sl = slice(hq4 * QN, (hq4 + 1) * QN)
                ex = pb.tile([32, QN], F32, tag="ex")
                nc.scalar.activation(ex[:16, :], DLa_t[:, sl], AF.Exp)
                rc = pb.tile([2, QN], F32, tag="rc")
                for w in range(QN // 512):
                    s_ps = pbp.tile([2, 512], F32, tag="sps")
                    nc.tensor.matmul(s_ps[:], Es_t[:], ex[:16, w*512:(w+1)*512],
                                     start=True, stop=True)
                    nc.vector.reciprocal(rc[:, w*512:(w+1)*512], s_ps[:])
                nc.sync.dma_start(ex[16:18, :], rc[:])
                rr = pb.tile([32, QN], F32, tag="rr")
                nc.vector.stream_shuffle(
                    rr[:], ex[:], [16 + (i // 8) for i in range(16)] + list(range(16)))
                anb = pb.tile([16, QN], BF16, tag="anb")
                nc.vector.tensor_tensor(anb[:], ex[:16, :], rr[:16, :], ALU.mult)
                nc.sync.dma_start(An_d[:, sl], anb[:])

        DLastack.close()
        # ========== phase C: coordinate pipeline per level ==========
        # packed: partition p=(m_loc,h24,hp), free sc=(k,w,h4b)  [128,1024]
        with tc.tile_pool(name="phC", bufs=1) as pc:
            phix_t = pc.tile([C, 2048], F32, tag="phx")
            nc.sync.dma_start(phix_t[:], phix_d)
            phiy_t = pc.tile([C, 2048], F32, tag="phy")
            nc.sync.dma_start(phiy_t[:], phiy_d)
            for l, (hl, wl) in enumerate(GRIDS):
                dxp = pc.tile([P, 1024], F32, tag="dxp")
                dyp = pc.tile([P, 1024], F32, tag="dyp")
                # dst col order (k, h4b, w): col = k*256 + hb*128 + w
                for mloc in range(2):
                    for k in range(K):
                        row = mloc * 16 + l * 8 + k * 2
                        for h24 in range(4):
                            pd = mloc * 64 + h24 * 16
                            for xy, dst in ((0, dxp), (1, dyp)):
                                v = DLd_t[row+xy:row+xy+1, :].rearrange(
                                    "r (hb hp hq w) -> r hb hp hq w",
                                    hb=2, hp=16, hq=4, w=128)
                                for hbv in range(2):
                                    vs = v[:, hbv:hbv+1, :, h24:h24+1, :]\
                                        .rearrange("r hb hp hq w -> r (hb hp hq) w")
                                    nc.sync.dma_start(
                                        dst[pd:pd+16,
                                            k*256+hbv*128:k*256+(hbv+1)*128],
                                        vs)
                csx = float(wl) / (wl - 1.0)
                csy = float(hl) / (hl - 1.0)
                # read dxp with (k, w, hb) order to land in sc order
                def sc_view(t):
                    return t[:].rearrange("p (k hb w) -> p k w hb",
                                          k=4, hb=2, w=128)
                ix = pc.tile([P, 1024], F32, tag="ix")
                ixv = ix[:].rearrange("p (k w hb) -> p k w hb", k=4, w=128, hb=2)
                nc.vector.tensor_tensor(
                    ixv, phix_t[:, l*1024:(l+1)*1024]
                    .rearrange("p (k w hb) -> p k w hb", k=4, w=128, hb=2),
                    sc_view(dxp), ALU.add)
                nc.scalar.activation(ix[:], ix[:], AF.Copy, scale=csx, bias=-0.5)
                iy = pc.tile([P, 1024], F32, tag="iy")
                iyv = iy[:].rearrange("p (k w hb) -> p k w hb", k=4, w=128, hb=2)
                nc.vector.tensor_tensor(
                    iyv, phiy_t[:, l*1024:(l+1)*1024]
                    .rearrange("p (k w hb) -> p k w hb", k=4, w=128, hb=2),
                    sc_view(dyp), ALU.add)
                nc.scalar.activation(iy[:], iy[:], AF.Copy, scale=csy, bias=-0.5)

                def floor_clamp(src, hi, tg):
                    sh = pc.tile([P, 1024], F32, tag="fcs")
                    nc.scalar.activation(sh[:], src[:], AF.Copy, bias=-0.5)
                    ii = pc.tile([P, 1024], I32, tag="fci")
                    nc.vector.tensor_copy(ii[:], sh[:])
                    ff = pc.tile([P, 1024], F32, tag="fcf")
                    nc.vector.tensor_copy(ff[:], ii[:])
                    c0 = pc.tile([P, 1024], F32, tag="fc0")
                    nc.vector.tensor_scalar_max(c0[:], ff[:], 0.0)
                    cc = pc.tile([P, 1024], F32, tag=tg)
                    nc.vector.tensor_scalar_min(cc[:], c0[:], float(hi))
                    return cc

                cx = floor_clamp(ix, wl - 2, "cx")
                cy = floor_clamp(iy, hl - 2, "cy")

                def tents(i_t, c_t, tg):
                    tt = pc.tile([P, 1024], F32, tag="tt")
                    nc.vector.tensor_tensor(tt[:], i_t[:], c_t[:], ALU.subtract)
                    pair = pc.tile([P, 2048], F32, tag=tg)
                    pv = pair[:].rearrange("p (n f) -> p n f", f=2)
                    ng = pc.tile([P, 1024], F32, tag="tng")
                    nc.vector.tensor_scalar(ng[:], tt[:], -1.0, None, ALU.mult)
                    a0 = pc.tile([P, 1024], F32, tag="ta0")
                    nc.vector.tensor_tensor(a0[:], tt[:], ng[:], ALU.max)
                    nc.scalar.activation(
                        pv[:, :, 0],
                        a0[:].rearrange("p (n o) -> p n o", o=1)[:, :, 0],
                        AF.Relu, scale=-1.0, bias=1.0)
                    a1 = pc.tile([P, 1024], F32, tag="ta1")
                    nc.vector.tensor_scalar(a1[:], tt[:], -1.0, None, ALU.add)
                    ng1 = pc.tile([P, 1024], F32, tag="tng1")
                    nc.vector.tensor_scalar(ng1[:], a1[:], -1.0, None, ALU.mult)
                    nc.vector.tensor_tensor(a1[:], a1[:], ng1[:], ALU.max)
                    nc.scalar.activation(
                        pv[:, :, 1],
                        a1[:].rearrange("p (n o) -> p n o", o=1)[:, :, 0],
                        AF.Relu, scale=-1.0, bias=1.0)
                    return pair

                u_t = tents(ix, cx, "u")
                v_t = tents(iy, cy, "v")

                idxf = pc.tile([P, 1024], F32, tag="idxf")
                nc.scalar.activation(idxf[:], cy[:], AF.Copy, scale=float(wl))
                nc.vector.tensor_tensor(idxf[:], idxf[:], cx[:], ALU.add)
                nc.vector.tensor_copy(idx_t[l][:], idxf[:])

                wt = w4_t[l]
                wv = wt[:].rearrange("p (n a b) -> p n a b", a=2, b=2)
                nc.vector.tensor_tensor(
                    wv,
                    v_t[:].rearrange("p (n a o) -> p n a o", a=2, o=1)
                        .broadcast_to((P, 1024, 2, 2)),
                    u_t[:].rearrange("p (n o b) -> p n o b", o=1, b=2)
                        .broadcast_to((P, 1024, 2, 2)),
                    ALU.mult)
        DLstack.close()

        # ========== phase D: x projections -> imgs -> DRAM ==========
        for l, (hl, wl) in enumerate(GRIDS):
            ncell = hl * wl
            img_d = img0_d if l == 0 else img1_d
            with tc.tile_pool(name="phD", bufs=2) as pd_, \
                 tc.tile_pool(name="phDp", bufs=2, space="PSUM") as pdp:
                Wp2_t = pd_.tile([C, 32], F32, tag="wp")
                nc.sync.dma_start(Wp2_t[:], Wp2_d)
                bp2_t = pd_.tile([32, 1], F32, tag="bp")
                nc.sync.dma_start(bp2_t[:], bp2_d)
                xd = x0T_d if l == 0 else x1T_d
                for w in range(ncell // 512):
                    xw = pd_.tile([C, 512], F32, tag="xw")
                    nc.sync.dma_start(xw[:], xd[:, w*512:(w+1)*512])
                    ip = pdp.tile([32, 512], F32, tag="ip")
                    nc.tensor.matmul(ip[:], Wp2_t[:], xw[:], start=True, stop=True)
                    ib = pd_.tile([32, 512], BF16, tag="ib")
                    nc.scalar.activation(ib[:], ip[:], AF.Identity, bias=bp2_t[:])
                    nc.sync.dma_start(img_d[:, w*512:(w+1)*512], ib[:])
                zb = pd_.tile([32, wl + 4], BF16, tag="zb")
                nc.vector.memset(zb[:], 0.0)
                nc.sync.dma_start(img_d[:, ncell:], zb[:])

        # ========== phase E: per-level sweep ==========
        for l, (hl, wl) in enumerate(GRIDS):
            ncell = hl * wl
            img_d = img0_d if l == 0 else img1_d
            with tc.tile_pool(name="src3", bufs=1) as ps3, \
                 tc.tile_pool(name="swp", bufs=2) as sw, \
                 tc.tile_pool(name="swp1", bufs=1) as sw1, \
                 tc.tile_pool(name="swpp", bufs=2, space="PSUM") as swp:
                s3 = ps3.tile([P, ncell * 4], BF16)
                s3v = s3[:].rearrange("p (n f) -> p n f", f=4)
                for mloc in range(2):
                    for h24 in range(4):
                        p0 = mloc * 64 + h24 * 16
                        for t in range(4):
                            off = (t // 2) * wl + (t % 2)
                            nc.sync.dma_start(
                                s3v[p0:p0+16, :, t],
                                img_d[mloc*16:(mloc+1)*16, off:off+ncell]
                                .rearrange("p (n o) -> p n o", o=1))
                for w in range(NWIN):
                    anw = sw.tile([16, WIN], BF16, tag="anw")
                    nc.sync.dma_start(anw[:], An_d[:, w*WIN:(w+1)*WIN])
                    an_ps = swp.tile([C, WIN], F32, tag="anps")
                    for q in range(WIN // 512):
                        nc.tensor.matmul(an_ps[:, q*512:(q+1)*512], Ra_t[:],
                                         anw[:, q*512:(q+1)*512],
                                         start=True, stop=True)
                    g_t = sw.tile([P, WIN * 4], BF16, tag="g")
                    nc.gpsimd.ap_gather(
                        g_t[:], s3[:], idx_t[l][:, w*(WIN//16):(w+1)*(WIN//16)],
                        channels=P, num_elems=ncell, d=4, num_idxs=WIN)
                    gv = g_t[:].rearrange("p (n hp f) -> p n hp f", hp=16, f=4)
                    for hp in range(16):
                        wsh = sw.tile([P, (WIN // 16) * 4], BF16, tag="wsh")
                        nc.vector.stream_shuffle(
                            wsh[:], w4_t[l][:, w*(WIN//16)*4:(w+1)*(WIN//16)*4],
                            [(i // 16) * 16 + hp for i in range(32)])
                        nc.vector.tensor_tensor(
                            gv[:, :, hp, :], gv[:, :, hp, :],
                            wsh[:].rearrange("p (n f) -> p n f", f=4), ALU.mult)
                    v1 = sw1.tile([P, WIN], F32, tag="v1")
                    nc.vector.tensor_reduce(
                        v1[:], g_t[:].rearrange("p (n f) -> p n f", f=4),
                        axis=AX.X, op=ALU.add)
                    v2 = sw1.tile([P, WIN], BF16, tag="v2")
                    nc.vector.tensor_tensor(v2[:], v1[:], an_ps[:], ALU.mult)
                    o_ps = swp.tile([C, WIN], F32, tag="ops")
                    for q in range(WIN // 512):
                        nc.tensor.matmul(o_ps[:, q*512:(q+1)*512], F_t[l][:],
                                         v2[:, q*512:(q+1)*512],
                                         start=True, stop=True)
                    ow = sw1.tile([C, WIN], F32, tag="ow")
                    if l == 0:
                        nc.scalar.activation(ow[:], o_ps[:], AF.Identity, bias=bm_t[:])
                        nc.sync.dma_start(part_d[:, w*WIN:(w+1)*WIN], ow[:])
                    else:
                        pw = sw.tile([C, WIN], F32, tag="pw")
                        nc.sync.dma_start(pw[:], part_d[:, w*WIN:(w+1)*WIN])
                        nc.vector.tensor_tensor(ow[:], o_ps[:], pw[:], ALU.add)
                        nc.sync.dma_start(out_d[:, w*WIN:(w+1)*WIN], ow[:])

    nc.compile()
    return nc


def _host_prep(z_q, x0, x1, p_q, Wq, bq, Wd, bd, Wa, ba, Wp, bp, Wm, bm):
    f32 = np.float32
    Wqd_r = (Wq @ Wd).astype(f32).reshape(C, M, L, K, 2)
    bqd_r = (bq @ Wd + bd).astype(f32).reshape(M, L, K, 2)
    Wqa_r = (Wq @ Wa).astype(f32).reshape(C, M, L * K)
    bqa_r = (bq @ Wa + ba).astype(f32).reshape(M, L * K)
    Wp_r = Wp.reshape(C, M, C_v)
    bp_r = bp.reshape(M, C_v)

    # packed coordinate helpers
    pml = np.arange(128) // 64
    ph24 = (np.arange(128) // 16) % 4
    php = np.arange(128) % 16
    kk = np.arange(1024) // 256
    ww = (np.arange(1024) // 2) % 128
    hb = np.arange(1024) % 2
    hq = (hb[None, :] * 16 + php[:, None]) * 4 + ph24[:, None]      # [128,1024]
    wq = np.broadcast_to(ww[None, :], (128, 1024))

    maps = []
    for c in range(N_CORES):
        b = c // 4
        m0 = 2 * (c % 4)
        Wc = np.zeros((C, 48), f32)
        bc = np.zeros((48, 1), f32)
        for ml in range(2):
            m = m0 + ml
            Wc[:, ml*16:(ml+1)*16] = Wqd_r[:, m].reshape(C, 16)
            bc[ml*16:(ml+1)*16, 0] = bqd_r[m].reshape(16)
            Wc[:, 32+ml*8:32+(ml+1)*8] = Wqa_r[:, m]
            bc[32+ml*8:32+(ml+1)*8, 0] = bqa_r[m]
        Wp2 = np.concatenate([Wp_r[:, m0], Wp_r[:, m0+1]], axis=1).astype(f32)
        bp2 = np.concatenate([bp_r[m0], bp_r[m0+1]])[:, None].astype(f32)
        Fs = []
        for l in range(2):
            Fl = np.zeros((C, C), f32)
            for p in range(128):
                ml, h24, j, s = p // 64, (p // 16) % 4, (p % 16) // 8, p % 8
                d2 = h24 * 4 + l * 2 + j
                Fl[p] = Wm[(m0 + ml) * C_v + d2]
            Fs.append(Fl)
        phix = np.zeros((C, 2048), f32)
        phiy = np.zeros((C, 2048), f32)
        par = (m0 + pml) % 2
        for l, (hl, wl) in enumerate(GRIDS):
            pq = p_q[par[:, None], hq, wq]
            phix[:, l*1024:(l+1)*1024] = pq[..., 0] * (wl - 1.0)
            phiy[:, l*1024:(l+1)*1024] = pq[..., 1] * (hl - 1.0)
        lead = (c % 4) == 0
        maps.append(dict(
            zqT=np.ascontiguousarray(z_q[b].reshape(HW, C).T),
            x0T=np.ascontiguousarray(x0[b].reshape(-1, C).T),
            x1T=np.ascontiguousarray(x1[b].reshape(-1, C).T),
            Wcmb=Wc, bcmb=bc, Wp2=Wp2, bp2=bp2,
            F0=Fs[0], F1=Fs[1], phix=phix, phiy=phiy,
            bmv=(bm[:, None].astype(f32) if lead else np.zeros((C, 1), f32)),
        ))
    return maps


def _install_err_capture():
    import traceback, subprocess
    from concourse import bass2jax as b2j
    orig = b2j.neuronx_cc_hook
    def wrapped(*a, **k):
        try:
            return orig(*a, **k)
        except BaseException as e:
            with open("/tmp/ncc_hook_err.txt", "w") as f:
                f.write(traceback.format_exc())
                ee = e
                while ee is not None:
                    if isinstance(ee, subprocess.CalledProcessError):
                        so = ee.stdout if isinstance(ee.stdout, str) else (ee.stdout or b"").decode(errors="replace")
                        f.write("\n==== STDOUT-tail ====\n" + so[-4000:])
                    ee = ee.__cause__ or ee.__context__
            raise
    b2j.neuronx_cc_hook = wrapped
    import libneuronxla
    libneuronxla.neuronx_cc = wrapped


def kernel(**inputs):
    _install_err_capture()
    maps = _host_prep(**{k: np.asarray(v) for k, v in inputs.items()})
    if "nc" not in _CACHED:
        _CACHED["nc"] = _build_program()
    res = run_bass_kernel_spmd(_CACHED["nc"], maps, list(range(N_CORES)))
    out = np.zeros((B, H, W, C), np.float32)
    for c in range(N_CORES):
        out[c // 4] += res.results[c]["outp"].T.reshape(H, W, C)
    return out



# revision 22
# speedup vs baseline: 3.8672x; 3.8672x over previous
"""Deformable head attention on 8 Trainium2 NeuronCores (Bass/Tile).

Sharding: core c -> batch b = c//4, heads (2*(c%4), 2*(c%4)+1).
Each core computes its two heads' contribution for all HW queries; the
output is ReduceScatter-summed on device over each 4-core batch group, so
core c returns the fully-reduced output rows [g*4096, (g+1)*4096) of its
batch (g = c%4).

Device layout: partition p = (mloc:2, kg:4, c:16). Partition (mloc,kg,c)
handles head mloc, sample point kg, image-channel c, and holds gather
indices/weights for queries q === c (mod 16) (element u = q//16). With the
ap_gather wrap rule (out col i <- idx partition i%16, element i//16) the
gather output columns land in natural query order, so the An multiply and
the final Wm matmul (which also sums heads, channels and K points across
partitions) need no further rearranging.

Bilinear taps come from two pair-planes: s2[:, 0:ncell] = img[0:ncell]
(pairs at even offsets) and s2[:, ncell:2ncell] = img[1:ncell+1] (odd
offsets) -- both contiguous copies. A d=2 gather with
idx = n>>1 + (n&1)*(ncell>>1) fetches (img[n], img[n+1]); top/bot row taps
are interleaved per-partition into one 2048-idx gather.
"""
import sys
import numpy as np
import ml_dtypes

sys.path.insert(0, "/opt/trn_rl_repo")

import concourse.bass as bass
import concourse.bacc as bacc
import concourse.tile as tile
from concourse import mybir
from contextlib import ExitStack

F32 = mybir.dt.float32
F16 = mybir.dt.float16
BF16 = mybir.dt.bfloat16
I16 = mybir.dt.int16
I32 = mybir.dt.int32
AF = mybir.ActivationFunctionType
ALU = mybir.AluOpType
AX = mybir.AxisListType

M, K, L, C = 8, 4, 2, 128
C_v = C // M
B, H, W = 2, 128, 128
HW = H * W
GRIDS = [(64, 64), (128, 128)]
N_CORES = 8
NWIN = 16
WIN = HW // NWIN  # 1024

_CACHED = {}
_DEBUG = False


def _build_program():
    P = 128
    nc = bacc.Bacc("TRN2", target_bir_lowering=False, debug=False,
                   num_devices=N_CORES)

    def I(name, shape, dt):
        return nc.dram_tensor(name, shape, dt, kind="ExternalInput").ap()

    zq_d = I("zq", [HW, C], BF16)
    x0_d = I("x0", [64 * 64, C], BF16)
    x1_d = I("x1", [HW, C], BF16)
    pqx_d = I("pqx", [128, 1024], F32)   # p_q x at (partition, slot) layout
    pqy_d = I("pqy", [128, 1024], F32)
    Wcmb_d = I("Wcmb", [C, 48], BF16)
    bcd_d = I("bcd", [32, 1], F32)
    bca_d = I("bca", [16, 1], F32)
    Wp2_d = I("Wp2", [C, 32], BF16)
    bp2_d = I("bp2", [32, 1], F32)
    F0_d = I("F0", [C, C], BF16)
    F1_d = I("F1", [C, C], BF16)
    bmv_d = I("bmv", [C, 1], F32)
    outp_d = nc.dram_tensor("outp", [HW // 4, C], F32,
                            kind="ExternalOutput").ap()

    dbg = {}
    if _DEBUG:
        for nm, shp, dt in [("dbgDLd", [32, HW], F16),
                            ("dbgAn", [16, HW], BF16),
                            ("dbgidx0", [128, 2048], I16),
                            ("dbgidx1", [128, 2048], I16),
                            ("dbgw40", [128, 4096], BF16),
                            ("dbgw41", [128, 4096], BF16),
                            ("dbgoutT", [HW, C], F32),
                            ("dbgdxp0", [128, 1024], F16),
                            ("dbgdyp0", [128, 1024], F16),
                            ("dbgphx", [128, 1024], F32)]:
            dbg[nm] = nc.dram_tensor(nm, shp, dt, kind="ExternalOutput").ap()
    An_d = nc.dram_tensor("An_i", [16, HW], BF16).ap()
    DLd_d = nc.dram_tensor("DLd_i", [32, HW], F16).ap()
    outT_d = nc.dram_tensor("outT_i", [HW, C], F32).ap()
    rs_d = nc.dram_tensor("rs_i", [HW // 4, C], F32).ap()

    # constants
    Ra = np.zeros((16, 128), np.float32)
    for p in range(128):
        Ra[(p // 64) * 8 + p % 8, p] = 1.0
    Es = np.zeros((16, 2), np.float32)
    for r in range(16):
        Es[r, r // 8] = 1.0
    eye_f = np.eye(128, dtype=np.float32)
    eye_b = np.eye(128, dtype=np.float32).astype(ml_dtypes.bfloat16)

    with tile.TileContext(nc) as tc, ExitStack() as ctx:
        const = ctx.enter_context(tc.tile_pool(name="const", bufs=1))
        eyeb_t = const.tile([128, 128], BF16, tag="eyeb")
        nc.sync.dma_start(eyeb_t[:], nc.inline_tensor(eye_b, name="eyeb").ap())
        eyef_t = const.tile([128, 128], F32, tag="eyef")
        nc.sync.dma_start(eyef_t[:], nc.inline_tensor(eye_f, name="eyef").ap())
        Ra_t = const.tile([16, 128], BF16, tag="Ra")
        nc.sync.dma_start(Ra_t[:], nc.inline_tensor(
            Ra.astype(ml_dtypes.bfloat16), name="Ra").ap())
        Es_t = const.tile([16, 2], F32, tag="Es")
        nc.sync.dma_start(Es_t[:], nc.inline_tensor(Es, name="Es").ap())
        Wcmb_t = const.tile([C, 48], BF16, tag="wcmb")
        nc.sync.dma_start(Wcmb_t[:], Wcmb_d)
        bcd_t = const.tile([32, 1], F32, tag="bcd")
        nc.sync.dma_start(bcd_t[:], bcd_d)
        bca_t = const.tile([16, 1], F32, tag="bca")
        nc.sync.dma_start(bca_t[:], bca_d)
        Wp2_t = const.tile([C, 32], BF16, tag="wp2")
        nc.sync.dma_start(Wp2_t[:], Wp2_d)
        bp2_t = const.tile([32, 1], F32, tag="bp2")
        nc.sync.dma_start(bp2_t[:], bp2_d)
        F_t = []
        for l, fd in enumerate([F0_d, F1_d]):
            ft = const.tile([C, C], BF16, tag=f"fm{l}")
            nc.sync.dma_start(ft[:], fd)
            F_t.append(ft)
        bmv_t = const.tile([C, 1], F32, tag="bmv")
        nc.sync.dma_start(bmv_t[:], bmv_d)

        # long-lived pipeline tensors (allocated up front: stack discipline)
        pipe = ctx.enter_context(tc.tile_pool(name="pipe", bufs=1))
        idxc_t = [pipe.tile([128, 2048], I16, tag=f"idxc{l}", name=f"idxc{l}")
                  for l in range(2)]
        W4c_t = [pipe.tile([128, 4096], BF16, tag=f"w4c{l}", name=f"w4c{l}")
                 for l in range(2)]
        s2_t = [pipe.tile([128, 2 * 64 * 64], BF16, tag="s20", name="s20"),
                pipe.tile([128, 2 * HW], BF16, tag="s21", name="s21")]

        DLa_stack = ExitStack()
        DLa_pool = DLa_stack.enter_context(tc.tile_pool(name="dla", bufs=1))
        DLa_t = DLa_pool.tile([16, HW], BF16, tag="dla")

        # ===== phase A: zT via PE transpose; delta/attn projections =====
        with tc.tile_pool(name="pa", bufs=2) as pa, \
             tc.tile_pool(name="pap", bufs=2, space="PSUM") as pap, \
             tc.tile_pool(name="pap48", bufs=2, space="PSUM") as pap48:
            for wi in range(NWIN):
                zrows = pa.tile([128, 1024], BF16, tag="zrows")
                nc.sync.dma_start(
                    zrows[:].rearrange("p (j c) -> p j c", j=8),
                    zq_d[wi * 1024:(wi + 1) * 1024, :]
                    .rearrange("(j p) c -> p j c", j=8))
                zT = pa.tile([128, 1024], BF16, tag="zT")
                for j in range(8):
                    tp = pap.tile([128, 128], BF16, tag="ztp")
                    nc.tensor.transpose(
                        tp[:], zrows[:, j * 128:(j + 1) * 128], eyeb_t[:])
                    nc.scalar.activation(zT[:, j * 128:(j + 1) * 128], tp[:],
                                         AF.Identity)
                p48 = pap48.tile([48, 1024], F32, tag="p48")
                for h in range(2):
                    nc.tensor.matmul(p48[:, h * 512:(h + 1) * 512], Wcmb_t[:],
                                     zT[:, h * 512:(h + 1) * 512],
                                     start=True, stop=True)
                sl = slice(wi * 1024, (wi + 1) * 1024)
                DLw = pa.tile([32, 1024], F16, tag="dlw")
                nc.scalar.activation(DLw[:], p48[:32, :], AF.Identity,
                                     bias=bcd_t[:])
                nc.sync.dma_start(DLd_d[:, sl], DLw[:])
                nc.scalar.activation(DLa_t[:, sl], p48[32:48, :], AF.Identity,
                                     bias=bca_t[:])

        # ===== phase B: softmax over lk=8 per head -> An_d (bf16) =====
        with tc.tile_pool(name="pb", bufs=1) as pb, \
             tc.tile_pool(name="pbp", bufs=2, space="PSUM") as pbp:
            QN = 2048
            for hq in range(HW // QN):
                sl = slice(hq * QN, (hq + 1) * QN)
                ex = pb.tile([32, QN], F32, tag="ex")
                nc.scalar.activation(ex[:16, :], DLa_t[:, sl], AF.Exp)
                rc = pb.tile([2, QN], F32, tag="rc")
                for w in range(QN // 512):
                    sps = pbp.tile([2, 512], F32, tag="sps")
                    nc.tensor.matmul(sps[:], Es_t[:],
                                     ex[:16, w * 512:(w + 1) * 512],
                                     start=True, stop=True)
                    nc.vector.reciprocal(rc[:, w * 512:(w + 1) * 512], sps[:])
                nc.sync.dma_start(ex[16:18, :], rc[:])
                rr = pb.tile([32, QN], F32, tag="rr")
                nc.vector.stream_shuffle(
                    rr[:], ex[:],
                    [16 + (i // 8) for i in range(16)] + list(range(16)))
                anb = pb.tile([16, QN], BF16, tag="anb")
                nc.vector.tensor_tensor(anb[:], ex[:16, :], rr[:16, :],
                                        ALU.mult)
                nc.sync.dma_start(An_d[:, sl], anb[:])
        DLa_stack.close()

        # ===== phase C: coordinates -> idxc, W4c (per level) =====
        DLdv = DLd_d.rearrange("(m l k x) q -> m l k x q", m=2, l=2, k=4,
                               x=2)
        for l, (hl, wl) in enumerate(GRIDS):
            ncell = hl * wl
            csx = float(wl) / (wl - 1.0)
            csy = float(hl) / (hl - 1.0)
            with tc.tile_pool(name="pc", bufs=1) as pc:
                dxp = pc.tile([128, 1024], F16, tag="dxp")
                dyp = pc.tile([128, 1024], F16, tag="dyp")
                # The whole coord pipeline runs in element order
                # u' = k*256 + up*128 + uw (dxp's DMA-natural layout).
                # Slot q' = u*16 + c with u = k*256 + uw*2 + up; sample
                # query qs = up*8192 + c*512 + h24*128 + uw.
                for mloc in range(2):
                    for xy, dst in ((0, dxp), (1, dyp)):
                        for k in range(4):
                            for h24 in range(4):
                                src = DLdv[mloc, l, k, xy, :].rearrange(
                                    "(up c h24 uw) -> h24 c up uw",
                                    up=2, c=16, h24=4, uw=128)[h24]
                                p0 = mloc * 64 + h24 * 16
                                nc.sync.dma_start(
                                    dst[p0:p0 + 16,
                                        k * 256:(k + 1) * 256]
                                    .rearrange("c (up uw) -> c up uw",
                                               up=2),
                                    src)
                phx = pc.tile([128, 1024], F32, tag="phx")
                nc.sync.dma_start(phx[:], pqx_d)
                phy = pc.tile([128, 1024], F32, tag="phy")
                nc.sync.dma_start(phy[:], pqy_d)
                if _DEBUG and l == 0:
                    nc.sync.dma_start(dbg["dbgdxp0"], dxp[:])
                    nc.sync.dma_start(dbg["dbgdyp0"], dyp[:])
                    nc.sync.dma_start(dbg["dbgphx"], phx[:])

                def coord(ph, dp, scale, cs, hi, itag, ctag):
                    t = pc.tile([128, 1024], F32, tag="s0")
                    nc.vector.tensor_scalar(t[:], ph[:], scale, None, ALU.mult)
                    nc.vector.tensor_tensor(t[:], t[:], dp[:], ALU.add)
                    ii = pc.tile([128, 1024], F32, tag=itag)
                    nc.scalar.activation(ii[:], t[:], AF.Copy, scale=cs,
                                         bias=-0.5)
                    # floor via round-nearest(v - 0.5) -> int32 -> f32
                    sh = pc.tile([128, 1024], F32, tag="s1")
                    nc.scalar.activation(sh[:], ii[:], AF.Copy, bias=-0.5)
                    iw = pc.tile([128, 1024], I32, tag="iw")
                    nc.vector.tensor_copy(iw[:], sh[:])
                    ff = pc.tile([128, 1024], F32, tag="s2s")
                    nc.vector.tensor_copy(ff[:], iw[:])
                    c0 = pc.tile([128, 1024], F32, tag="s3")
                    nc.vector.tensor_scalar_max(c0[:], ff[:], 0.0)
                    cc = pc.tile([128, 1024], F32, tag=ctag)
                    nc.vector.tensor_scalar_min(cc[:], c0[:], float(hi))
                    return ii, cc

                ix, cx = coord(phx, dxp, wl - 1.0, csx, wl - 2, "ix", "cx")
                iy, cy = coord(phy, dyp, hl - 1.0, csy, hl - 2, "iy", "cy")

                # idx_top = cy*(wl/2) + (cx>>1) + (cx&1)*(ncell/2)
                hf = pc.tile([128, 1024], F32, tag="s0")
                nc.scalar.activation(hf[:], cx[:], AF.Copy, scale=0.5,
                                     bias=-0.25)
                hfi = pc.tile([128, 1024], I32, tag="iw")
                nc.vector.tensor_copy(hfi[:], hf[:])
                nc.vector.tensor_copy(hf[:], hfi[:])
                par = pc.tile([128, 1024], F32, tag="s1")
                nc.vector.tensor_scalar(par[:], hf[:], -2.0, None, ALU.mult)
                nc.vector.tensor_tensor(par[:], par[:], cx[:], ALU.add)
                it = pc.tile([128, 1024], F32, tag="s2s")
                nc.scalar.activation(it[:], cy[:], AF.Copy, scale=wl / 2.0)
                nc.vector.tensor_tensor(it[:], it[:], hf[:], ALU.add)
                nc.vector.tensor_scalar(par[:], par[:], ncell / 2.0, None,
                                        ALU.mult)
                nc.vector.tensor_tensor(it[:], it[:], par[:], ALU.add)
                ib = pc.tile([128, 1024], F32, tag="s3")
                nc.vector.tensor_scalar(ib[:], it[:], wl / 2.0, None, ALU.add)
                # idxc window-major: col = wi*128 + up*64 + uwl*2 + tb
                # (wi = k*4 + uwq, uw = uwq*32 + uwl); it/ib are u'-ordered
                idv = idxc_t[l][:].rearrange(
                    "p (k uwq up uwl tb) -> p k tb uwq up uwl",
                    k=4, uwq=4, up=2, uwl=32, tb=2)
                for src_t, tb in ((it, 0), (ib, 1)):
                    sv = src_t[:].rearrange(
                        "p (k up uwq uwl) -> p k uwq up uwl",
                        k=4, up=2, uwq=4, uwl=32)
                    for k in range(4):
                        nc.vector.tensor_copy(idv[:, k, tb], sv[:, k])

                def tents(i_t, c_t, prtag):
                    tt = pc.tile([128, 1024], F32, tag="s0")
                    nc.vector.tensor_tensor(tt[:], i_t[:], c_t[:],
                                            ALU.subtract)
                    pair = pc.tile([128, 2048], F32, tag=prtag)
                    pv = pair[:].rearrange("p (n f) -> p n f", f=2)
                    ng = pc.tile([128, 1024], F32, tag="s1")
                    nc.vector.tensor_scalar(ng[:], tt[:], -1.0, None, ALU.mult)
                    a0 = pc.tile([128, 1024], F32, tag="s2s")
                    nc.vector.tensor_tensor(a0[:], tt[:], ng[:], ALU.max)
                    nc.scalar.activation(
                        pv[:, :, 0],
                        a0[:].rearrange("p (n o) -> p n o", o=1)[:, :, 0],
                        AF.Relu, scale=-1.0, bias=1.0)
                    a1 = pc.tile([128, 1024], F32, tag="s3")
                    nc.vector.tensor_scalar(a1[:], tt[:], -1.0, None, ALU.add)
                    ng1 = pc.tile([128, 1024], F32, tag="s1")
                    nc.vector.tensor_scalar(ng1[:], a1[:], -1.0, None,
                                            ALU.mult)
                    nc.vector.tensor_tensor(a1[:], a1[:], ng1[:], ALU.max)
                    nc.scalar.activation(
                        pv[:, :, 1],
                        a1[:].rearrange("p (n o) -> p n o", o=1)[:, :, 0],
                        AF.Relu, scale=-1.0, bias=1.0)
                    return pair

                u_pr = tents(ix, cx, "upr")
                v_pr = tents(iy, cy, "vpr")

                # W4c[p, (u, half, lr)] = v[half] * u[lr]
                w4v = W4c_t[l][:].rearrange("p (n a b) -> p n a b", a=2, b=2)
                nc.vector.tensor_tensor(
                    w4v,
                    v_pr[:].rearrange("p (n a o) -> p n a o", a=2, o=1)
                    .broadcast_to((P, 1024, 2, 2)),
                    u_pr[:].rearrange("p (n o b) -> p n o b", o=1, b=2)
                    .broadcast_to((P, 1024, 2, 2)),
                    ALU.mult)
        # ===== phase D: value projection -> img -> s2 pair planes =====
        for l, (hl, wl) in enumerate(GRIDS):
            ncell = hl * wl
            x_d = x0_d if l == 0 else x1_d
            with tc.tile_pool(name="pdi", bufs=1) as pdi, \
                 tc.tile_pool(name="pd", bufs=2) as pd_, \
                 tc.tile_pool(name="pdp", bufs=2, space="PSUM") as pdp, \
                 tc.tile_pool(name="pdp2", bufs=2, space="PSUM") as pdp2:
                img = pdi.tile([32, ncell + 8], BF16, tag="img")
                zpad = pd_.tile([32, 8], BF16, tag="zpad")
                nc.vector.memset(zpad[:], 0.0)
                nc.sync.dma_start(img[:, ncell:], zpad[:])
                for ch in range(ncell // 512):
                    xr = pd_.tile([128, 512], BF16, tag="xr")
                    nc.sync.dma_start(
                        xr[:].rearrange("p (j c) -> p j c", j=4),
                        x_d[ch * 512:(ch + 1) * 512, :]
                        .rearrange("(j p) c -> p j c", j=4))
                    xT = pd_.tile([128, 512], BF16, tag="xT")
                    for j in range(4):
                        tp = pdp.tile([128, 128], BF16, tag="xtp")
                        nc.tensor.transpose(
                            tp[:], xr[:, j * 128:(j + 1) * 128], eyeb_t[:])
                        nc.scalar.activation(xT[:, j * 128:(j + 1) * 128],
                                             tp[:], AF.Identity)
                    ips = pdp2.tile([32, 512], F32, tag="ips")
                    nc.tensor.matmul(ips[:], Wp2_t[:], xT[:], start=True,
                                     stop=True)
                    nc.scalar.activation(img[:, ch * 512:(ch + 1) * 512],
                                         ips[:], AF.Identity, bias=bp2_t[:])
                for mloc in range(2):
                    for kg in range(4):
                        g0 = mloc * 64 + kg * 16
                        rs = slice(mloc * 16, (mloc + 1) * 16)
                        nc.sync.dma_start(s2_t[l][g0:g0 + 16, :ncell],
                                          img[rs, :ncell])
                        nc.sync.dma_start(s2_t[l][g0:g0 + 16, ncell:],
                                          img[rs, 1:ncell + 1])

        # ===== phase E: gather, combine, project, transpose out =====
        with tc.tile_pool(name="pe", bufs=2) as pe, \
             tc.tile_pool(name="pe1", bufs=2) as pe1, \
             tc.tile_pool(name="pout", bufs=1, space="PSUM") as pout, \
             tc.tile_pool(name="pan", bufs=1, space="PSUM") as pan, \
             tc.tile_pool(name="ptp", bufs=2, space="PSUM") as ptp:
            for wi in range(NWIN):
                owin = pout.tile([128, 1024], F32, tag="owps")
                for l, (hl, wl) in enumerate(GRIDS):
                    ncell = hl * wl
                    anw = pe1.tile([16, 1024], BF16, tag="anw")
                    nc.sync.dma_start(anw[:],
                                      An_d[:, wi * 1024:(wi + 1) * 1024])
                    anp = pan.tile([128, 1024], F32, tag="anp")
                    for h in range(2):
                        nc.tensor.matmul(
                            anp[:, h * 512:(h + 1) * 512], Ra_t[:],
                            anw[:, h * 512:(h + 1) * 512],
                            start=True, stop=True)
                    g2 = pe.tile([128, 4096], BF16, tag="g2")
                    nc.gpsimd.ap_gather(
                        g2[:].rearrange("p (n d) -> p n d", d=2),
                        s2_t[l][:].rearrange("p (n d) -> p n d", d=2),
                        idxc_t[l][:, wi * 128:(wi + 1) * 128],
                        channels=128, num_elems=ncell, d=2, num_idxs=2048)
                    g2v = g2[:].rearrange("p (e tb c lr) -> p c e tb lr",
                                          e=64, tb=2, c=16, lr=2)
                    w4v = W4c_t[l][:].rearrange(
                        "p (k up uwq uwl hr) -> p k uwq up uwl hr",
                        k=4, up=2, uwq=4, uwl=32, hr=4)[:, wi // 4, wi % 4]
                    for cc in range(16):
                        wsh = pe1.tile([128, 256], BF16, tag="wsh")
                        nc.vector.stream_shuffle(
                            wsh[:].rearrange("p (up uwl hr) -> p up uwl hr",
                                             up=2, uwl=32),
                            w4v,
                            [(j // 16) * 16 + cc for j in range(32)])
                        gslice = g2v[:, cc:cc + 1, :, :, :].rearrange(
                            "p c e tb lr -> p (c e) tb lr")
                        nc.vector.tensor_tensor(
                            gslice, gslice,
                            wsh[:].rearrange("p (e h lr) -> p e h lr",
                                             e=64, h=2, lr=2),
                            ALU.mult)
                    r1 = pe1.tile([128, 2048], F32, tag="r1")
                    nc.vector.tensor_reduce(
                        r1[:], g2[:].rearrange("p (n lr) -> p n lr", lr=2),
                        axis=AX.X, op=ALU.add)
                    r1v = r1[:].rearrange(
                        "p (up uwl tb c) -> p tb up uwl c",
                        up=2, uwl=32, tb=2, c=16)
                    va = pe1.tile([128, 1024], F32, tag="va")
                    nc.vector.tensor_tensor(
                        va[:].rearrange("p (uwl up c) -> p up uwl c",
                                        uwl=32, up=2, c=16),
                        r1v[:, 0], r1v[:, 1], ALU.add)
                    v2 = pe1.tile([128, 1024], BF16, tag="v2")
                    nc.vector.tensor_tensor(v2[:], va[:], anp[:], ALU.mult)
                    for h in range(2):
                        nc.tensor.matmul(owin[:, h * 512:(h + 1) * 512],
                                         F_t[l][:],
                                         v2[:, h * 512:(h + 1) * 512],
                                         start=(l == 0), stop=(l == 1))
                ow = pe.tile([128, 1024], F32, tag="owsb")
                nc.scalar.activation(ow[:], owin[:], AF.Identity,
                                     bias=bmv_t[:])
                outw = pe.tile([128, 1024], F32, tag="outw")
                for j in range(8):
                    tp = ptp.tile([128, 128], F32, tag="otp")
                    nc.tensor.transpose(tp[:], ow[:, j * 128:(j + 1) * 128],
                                        eyef_t[:])
                    nc.vector.tensor_copy(outw[:, j * 128:(j + 1) * 128],
                                          tp[:])
                nc.sync.dma_start(
                    outT_d[wi * 1024:(wi + 1) * 1024, :]
                    .rearrange("(j p) c -> p j c", j=8),
                    outw[:].rearrange("p (j c) -> p j c", j=8))
            if _DEBUG:
                nc.sync.dma_start(dbg["dbgDLd"], DLd_d)
                nc.sync.dma_start(dbg["dbgAn"], An_d)
                nc.sync.dma_start(dbg["dbgidx0"], idxc_t[0][:])
                nc.sync.dma_start(dbg["dbgidx1"], idxc_t[1][:])
                nc.sync.dma_start(dbg["dbgw40"], W4c_t[0][:])
                nc.sync.dma_start(dbg["dbgw41"], W4c_t[1][:])
                nc.sync.dma_start(dbg["dbgoutT"], outT_d)
            nc.gpsimd.collective_compute(
                "ReduceScatter", ALU.add,
                replica_groups=[[0, 1, 2, 3], [4, 5, 6, 7]],
                ins=[outT_d], outs=[rs_d])
            nc.sync.dma_start(outp_d, rs_d)

    nc.compile()
    return nc


def _to_bf16(a):
    """Fast f32 -> bf16 with round-to-nearest-even."""
    u = np.ascontiguousarray(a, dtype=np.float32).view(np.uint32)
    r = ((u >> 16) & np.uint32(1)) + np.uint32(0x7FFF)
    return ((u + r) >> np.uint32(16)).astype(np.uint16).view(
        ml_dtypes.bfloat16)


_PIDX = np.arange(128)
_MLOC = _PIDX // 64
_H24 = (_PIDX // 16) % 4
# element order u' = k*256 + up*128 + uw; sample query
# qs = ((up*16 + c)*4 + h24)*128 + uw
_UP = (np.arange(1024) // 128) % 2
_UW = np.arange(1024) % 128
_QS = (((_UP[None, :] * 16 + (_PIDX % 16)[:, None]) * 4
        + _H24[:, None]) * 128 + _UW[None, :])


def _host_prep(z_q, x0, x1, p_q, Wq, bq, Wd, bd, Wa, ba, Wp, bp, Wm, bm):
    f32 = np.float32
    Wqd_r = (Wq @ Wd).astype(f32).reshape(C, M, L, K, 2)
    bqd_r = (bq @ Wd + bd).astype(f32).reshape(M, L, K, 2)
    Wqa_r = (Wq @ Wa).astype(f32).reshape(C, M, L * K)
    bqa_r = (bq @ Wa + ba).astype(f32).reshape(M, L * K)
    Wp_r = Wp.reshape(C, M, C_v)
    bp_r = bp.reshape(M, C_v)

    # p_q gathered into the device (partition, slot) layout: partition
    # p = (mloc, h24, c), slot q' = u*16 + c, sample query
    # qs = ((u%2)*16 + c)*512 + h24*128 + (u//2)%128 -- implements the
    # reference's faithful scrambled permute/view pairing. phi uses
    # p_q[m % 2] = p_q[mloc] (faithful m*B+b vs b*M+m batch mismatch).
    pq = np.asarray(p_q, f32).reshape(2, HW, 2)
    pqx = pq[_MLOC[:, None], _QS, 0]
    pqy = pq[_MLOC[:, None], _QS, 1]

    zb = [_to_bf16(np.asarray(z_q[b]).reshape(HW, C)) for b in range(B)]
    x0b = [_to_bf16(np.asarray(x0[b]).reshape(-1, C)) for b in range(B)]
    x1b = [_to_bf16(np.asarray(x1[b]).reshape(-1, C)) for b in range(B)]

    maps = []
    for c in range(N_CORES):
        b = c // 4
        m0 = 2 * (c % 4)
        Wc = np.zeros((C, 48), f32)
        bcd = np.zeros((32, 1), f32)
        bca = np.zeros((16, 1), f32)
        for ml in range(2):
            m = m0 + ml
            Wc[:, ml * 16:(ml + 1) * 16] = Wqd_r[:, m].reshape(C, 16)
            bcd[ml * 16:(ml + 1) * 16, 0] = bqd_r[m].reshape(16)
            Wc[:, 32 + ml * 8:32 + (ml + 1) * 8] = Wqa_r[:, m]
            bca[ml * 8:(ml + 1) * 8, 0] = bqa_r[m]
        Wp2 = np.concatenate([Wp_r[:, m0], Wp_r[:, m0 + 1]], axis=1)
        bp2 = np.concatenate([bp_r[m0], bp_r[m0 + 1]])[:, None].astype(f32)
        Fs = []
        for l in range(2):
            rows = (m0 + _MLOC) * C_v + _H24 * 4 + l * 2 + (_PIDX % 16) // 8
            Fs.append(_to_bf16(Wm[rows].astype(f32)))
        lead = (c % 4) == 0
        maps.append(dict(
            zq=zb[b], x0=x0b[b], x1=x1b[b], pqx=pqx, pqy=pqy,
            Wcmb=_to_bf16(Wc), bcd=bcd, bca=bca,
            Wp2=_to_bf16(Wp2.astype(f32)), bp2=bp2, F0=Fs[0], F1=Fs[1],
            bmv=(np.asarray(bm, f32)[:, None].copy() if lead
                 else np.zeros((C, 1), f32)),
        ))
    return maps


def _install_err_capture():
    import traceback, subprocess
    from concourse import bass2jax as b2j
    if getattr(b2j, "_err_capture_installed", False):
        return
    orig = b2j.neuronx_cc_hook

    def wrapped(*a, **k):
        try:
            return orig(*a, **k)
        except BaseException as e:
            with open("/tmp/ncc_hook_err.txt", "w") as f:
                f.write(traceback.format_exc())
                ee = e
                while ee is not None:
                    if isinstance(ee, subprocess.CalledProcessError):
                        so = ee.stdout if isinstance(ee.stdout, str) else (
                            ee.stdout or b"").decode(errors="replace")
                        f.write("\n==== STDOUT-tail ====\n" + so[-4000:])
                    ee = ee.__cause__ or ee.__context__
            raise

    b2j.neuronx_cc_hook = wrapped
    b2j._err_capture_installed = True
    import libneuronxla
    libneuronxla.neuronx_cc = wrapped


class CachedRunner:
    """Build the shard_map jit wrapper for a Bass program once and reuse it
    for every call (run_bass_kernel_spmd rebuilds and retraces per call)."""

    def __init__(self, nc, n_cores=N_CORES):
        import jax
        from jax.sharding import Mesh, PartitionSpec
        from jax.experimental.shard_map import shard_map
        from concourse.bass2jax import (
            _bass_exec_p, partition_id_tensor, install_neuronx_cc_hook)
        install_neuronx_cc_hook()
        self.nc = nc
        self.n_cores = n_cores
        partition_name = (nc.partition_id_tensor.name
                          if nc.partition_id_tensor else None)
        in_names, out_names, out_avals, zero_shapes = [], [], [], []
        for alloc in nc.m.functions[0].allocations:
            if not isinstance(alloc, mybir.MemoryLocationSet):
                continue
            name = alloc.memorylocations[0].name
            if alloc.kind == "ExternalInput":
                if name != partition_name:
                    in_names.append(name)
            elif alloc.kind == "ExternalOutput":
                shape = tuple(alloc.tensor_shape)
                dtype = mybir.dt.np(alloc.dtype)
                out_avals.append(jax.core.ShapedArray(shape, dtype))
                out_names.append(name)
                zero_shapes.append((shape, dtype))
        self.in_names = list(in_names)
        self.out_names = out_names
        self.out_avals = out_avals
        self.zero_shapes = zero_shapes
        n_params = len(in_names)
        n_outs = len(out_avals)
        all_names = list(in_names) + list(out_names)
        if partition_name is not None:
            all_names.append(partition_name)
        donate = tuple(range(n_params, n_params + n_outs))

        def _body(*args):
            operands = list(args)
            if partition_name is not None:
                operands.append(partition_id_tensor())
            outs = _bass_exec_p.bind(
                *operands,
                out_avals=tuple(out_avals),
                in_names=tuple(all_names),
                out_names=tuple(out_names),
                lowering_input_output_aliases=(),
                sim_require_finite=True,
                sim_require_nnan=True,
                nc=nc,
            )
            return tuple(outs)

        devices = jax.devices()[:n_cores]
        mesh = Mesh(np.asarray(devices), ("core",))
        in_specs = (PartitionSpec("core"),) * (n_params + n_outs)
        out_specs = (PartitionSpec("core"),) * n_outs
        self._fn = jax.jit(
            shard_map(_body, mesh=mesh, in_specs=in_specs,
                      out_specs=out_specs, check_rep=False),
            donate_argnums=donate, keep_unused=True)

    def __call__(self, concat_inputs):
        """concat_inputs: arrays of shape (n_cores*dim0, ...) in in_names
        order. Returns list of np arrays (n_cores, *out_shape)."""
        zeros = [np.zeros((self.n_cores * s[0], *s[1:]), d)
                 for s, d in self.zero_shapes]
        outs = self._fn(*concat_inputs, *zeros)
        return [np.asarray(o).reshape(self.n_cores, *self.out_avals[i].shape)
                for i, o in enumerate(outs)]


def _concat_from_maps(runner, maps):
    return [np.concatenate([np.asarray(m[name]) for m in maps], axis=0)
            for name in runner.in_names]


def kernel(**inputs):
    _install_err_capture()
    maps = _host_prep(**{k: np.asarray(v) for k, v in inputs.items()})
    if "runner" not in _CACHED:
        _CACHED["nc"] = _build_program()
        _CACHED["runner"] = CachedRunner(_CACHED["nc"])
    runner = _CACHED["runner"]
    res = runner(_concat_from_maps(runner, maps))[0]  # [8, 4096, C]
    out = np.empty((B, H, W, C), np.float32)
    ov = out.reshape(B, 4, HW // 4, C)
    for c in range(N_CORES):
        ov[c // 4, c % 4] = res[c]
    return out


# revision 23
# speedup vs baseline: 12.4763x; 3.2262x over previous
"""Deformable head attention on 8 Trainium2 NeuronCores (Bass/Tile).

Sharding: core c -> batch b = c//4, heads (2*(c%4), 2*(c%4)+1).
Each core computes its two heads' contribution for all HW queries; the
output is ReduceScatter-summed on device over each 4-core batch group, so
core c returns the fully-reduced output rows [g*4096, (g+1)*4096) of its
batch (g = c%4).

Device layout: partition p = (mloc:2, kg:4, c:16). Partition (mloc,kg,c)
handles head mloc, sample point kg, image-channel c, and holds gather
indices/weights for queries q === c (mod 16) (element u = q//16). With the
ap_gather wrap rule (out col i <- idx partition i%16, element i//16) the
gather output columns land in natural query order, so the An multiply and
the final Wm matmul (which also sums heads, channels and K points across
partitions) need no further rearranging.

Bilinear taps come from two pair-planes: s2[:, 0:ncell] = img[0:ncell]
(pairs at even offsets) and s2[:, ncell:2ncell] = img[1:ncell+1] (odd
offsets) -- both contiguous copies. A d=2 gather with
idx = n>>1 + (n&1)*(ncell>>1) fetches (img[n], img[n+1]); top/bot row taps
are interleaved per-partition into one 2048-idx gather.
"""
import sys
import numpy as np
import ml_dtypes

sys.path.insert(0, "/opt/trn_rl_repo")

import concourse.bass as bass
import concourse.bacc as bacc
import concourse.tile as tile
from concourse import mybir
from contextlib import ExitStack

F32 = mybir.dt.float32
F16 = mybir.dt.float16
BF16 = mybir.dt.bfloat16
I16 = mybir.dt.int16
I32 = mybir.dt.int32
AF = mybir.ActivationFunctionType
ALU = mybir.AluOpType
AX = mybir.AxisListType

M, K, L, C = 8, 4, 2, 128
C_v = C // M
B, H, W = 2, 128, 128
HW = H * W
GRIDS = [(64, 64), (128, 128)]
N_CORES = 8
NWIN = 16
WIN = HW // NWIN  # 1024

_CACHED = {}
_DEBUG = False


def _build_program():
    P = 128
    nc = bacc.Bacc("TRN2", target_bir_lowering=False, debug=False,
                   num_devices=N_CORES)

    def I(name, shape, dt):
        return nc.dram_tensor(name, shape, dt, kind="ExternalInput").ap()

    zq_d = I("zq", [HW, C], BF16)
    x0_d = I("x0", [64 * 64, C], BF16)
    x1_d = I("x1", [HW, C], BF16)
    pqx_d = I("pqx", [128, 1024], F32)   # p_q x at (partition, slot) layout
    pqy_d = I("pqy", [128, 1024], F32)
    Wcmb_d = I("Wcmb", [C, 48], BF16)
    bcd_d = I("bcd", [32, 1], F32)
    bca_d = I("bca", [16, 1], F32)
    Wp2_d = I("Wp2", [C, 32], BF16)
    bp2_d = I("bp2", [32, 1], F32)
    F0_d = I("F0", [C, C], BF16)
    F1_d = I("F1", [C, C], BF16)
    bmv_d = I("bmv", [C, 1], F32)
    outp_d = nc.dram_tensor("outp", [HW // 4, C], F32,
                            kind="ExternalOutput").ap()

    dbg = {}
    if _DEBUG:
        for nm, shp, dt in [("dbgDLd", [32, HW], F16),
                            ("dbgAn", [16, HW], BF16),
                            ("dbgidx0", [128, 2048], I16),
                            ("dbgidx1", [128, 2048], I16),
                            ("dbgw40", [128, 4096], BF16),
                            ("dbgw41", [128, 4096], BF16),
                            ("dbgoutT", [HW, C], F32),
                            ("dbgdxp0", [128, 1024], F16),
                            ("dbgdyp0", [128, 1024], F16),
                            ("dbgphx", [128, 1024], F32)]:
            dbg[nm] = nc.dram_tensor(nm, shp, dt, kind="ExternalOutput").ap()
    An_d = nc.dram_tensor("An_i", [16, HW], BF16).ap()
    DLd_d = nc.dram_tensor("DLd_i", [32, HW], F16).ap()
    outT_d = nc.dram_tensor("outT_i", [HW, C], F32).ap()
    rs_d = nc.dram_tensor("rs_i", [HW // 4, C], F32).ap()

    # constants
    Ra = np.zeros((16, 128), np.float32)
    for p in range(128):
        Ra[(p // 64) * 8 + p % 8, p] = 1.0
    Es = np.zeros((16, 2), np.float32)
    for r in range(16):
        Es[r, r // 8] = 1.0
    eye_f = np.eye(128, dtype=np.float32)
    eye_b = np.eye(128, dtype=np.float32).astype(ml_dtypes.bfloat16)

    with tile.TileContext(nc) as tc, ExitStack() as ctx:
        const = ctx.enter_context(tc.tile_pool(name="const", bufs=1))
        eyeb_t = const.tile([128, 128], BF16, tag="eyeb")
        nc.sync.dma_start(eyeb_t[:], nc.inline_tensor(eye_b, name="eyeb").ap())
        eyef_t = const.tile([128, 128], F32, tag="eyef")
        nc.sync.dma_start(eyef_t[:], nc.inline_tensor(eye_f, name="eyef").ap())
        Ra_t = const.tile([16, 128], BF16, tag="Ra")
        nc.sync.dma_start(Ra_t[:], nc.inline_tensor(
            Ra.astype(ml_dtypes.bfloat16), name="Ra").ap())
        Es_t = const.tile([16, 2], F32, tag="Es")
        nc.sync.dma_start(Es_t[:], nc.inline_tensor(Es, name="Es").ap())
        Wcmb_t = const.tile([C, 48], BF16, tag="wcmb")
        nc.sync.dma_start(Wcmb_t[:], Wcmb_d)
        bcd_t = const.tile([32, 1], F32, tag="bcd")
        nc.sync.dma_start(bcd_t[:], bcd_d)
        bca_t = const.tile([16, 1], F32, tag="bca")
        nc.sync.dma_start(bca_t[:], bca_d)
        Wp2_t = const.tile([C, 32], BF16, tag="wp2")
        nc.sync.dma_start(Wp2_t[:], Wp2_d)
        bp2_t = const.tile([32, 1], F32, tag="bp2")
        nc.sync.dma_start(bp2_t[:], bp2_d)
        F_t = []
        for l, fd in enumerate([F0_d, F1_d]):
            ft = const.tile([C, C], BF16, tag=f"fm{l}")
            nc.sync.dma_start(ft[:], fd)
            F_t.append(ft)
        bmv_t = const.tile([C, 1], F32, tag="bmv")
        nc.sync.dma_start(bmv_t[:], bmv_d)

        # long-lived pipeline tensors (allocated up front: stack discipline)
        pipe = ctx.enter_context(tc.tile_pool(name="pipe", bufs=1))
        idxc_t = [pipe.tile([128, 2048], I16, tag=f"idxc{l}", name=f"idxc{l}")
                  for l in range(2)]
        W4c_t = [pipe.tile([128, 4096], BF16, tag=f"w4c{l}", name=f"w4c{l}")
                 for l in range(2)]
        s2_t = [pipe.tile([128, 2 * 64 * 64], BF16, tag="s20", name="s20"),
                pipe.tile([128, 2 * HW], BF16, tag="s21", name="s21")]

        DLa_stack = ExitStack()
        DLa_pool = DLa_stack.enter_context(tc.tile_pool(name="dla", bufs=1))
        DLa_t = DLa_pool.tile([16, HW], BF16, tag="dla")

        # ===== phase A: zT via PE transpose; delta/attn projections =====
        with tc.tile_pool(name="pa", bufs=2) as pa, \
             tc.tile_pool(name="pap", bufs=2, space="PSUM") as pap, \
             tc.tile_pool(name="pap48", bufs=2, space="PSUM") as pap48:
            for wi in range(NWIN):
                zrows = pa.tile([128, 1024], BF16, tag="zrows")
                nc.sync.dma_start(
                    zrows[:].rearrange("p (j c) -> p j c", j=8),
                    zq_d[wi * 1024:(wi + 1) * 1024, :]
                    .rearrange("(j p) c -> p j c", j=8))
                zT = pa.tile([128, 1024], BF16, tag="zT")
                for j in range(8):
                    tp = pap.tile([128, 128], BF16, tag="ztp")
                    nc.tensor.transpose(
                        tp[:], zrows[:, j * 128:(j + 1) * 128], eyeb_t[:])
                    nc.scalar.activation(zT[:, j * 128:(j + 1) * 128], tp[:],
                                         AF.Identity)
                p48 = pap48.tile([48, 1024], F32, tag="p48")
                for h in range(2):
                    nc.tensor.matmul(p48[:, h * 512:(h + 1) * 512], Wcmb_t[:],
                                     zT[:, h * 512:(h + 1) * 512],
                                     start=True, stop=True)
                sl = slice(wi * 1024, (wi + 1) * 1024)
                DLw = pa.tile([32, 1024], F16, tag="dlw")
                nc.scalar.activation(DLw[:], p48[:32, :], AF.Identity,
                                     bias=bcd_t[:])
                nc.sync.dma_start(DLd_d[:, sl], DLw[:])
                nc.scalar.activation(DLa_t[:, sl], p48[32:48, :], AF.Identity,
                                     bias=bca_t[:])

        # ===== phase B: softmax over lk=8 per head -> An_d (bf16) =====
        with tc.tile_pool(name="pb", bufs=1) as pb, \
             tc.tile_pool(name="pbp", bufs=2, space="PSUM") as pbp:
            QN = 2048
            for hq in range(HW // QN):
                sl = slice(hq * QN, (hq + 1) * QN)
                ex = pb.tile([32, QN], F32, tag="ex")
                nc.scalar.activation(ex[:16, :], DLa_t[:, sl], AF.Exp)
                rc = pb.tile([2, QN], F32, tag="rc")
                for w in range(QN // 512):
                    sps = pbp.tile([2, 512], F32, tag="sps")
                    nc.tensor.matmul(sps[:], Es_t[:],
                                     ex[:16, w * 512:(w + 1) * 512],
                                     start=True, stop=True)
                    nc.vector.reciprocal(rc[:, w * 512:(w + 1) * 512], sps[:])
                nc.sync.dma_start(ex[16:18, :], rc[:])
                rr = pb.tile([32, QN], F32, tag="rr")
                nc.vector.stream_shuffle(
                    rr[:], ex[:],
                    [16 + (i // 8) for i in range(16)] + list(range(16)))
                anb = pb.tile([16, QN], BF16, tag="anb")
                nc.vector.tensor_tensor(anb[:], ex[:16, :], rr[:16, :],
                                        ALU.mult)
                nc.sync.dma_start(An_d[:, sl], anb[:])
        DLa_stack.close()

        # ===== phase C: coordinates -> idxc, W4c (per level) =====
        DLdv = DLd_d.rearrange("(m l k x) q -> m l k x q", m=2, l=2, k=4,
                               x=2)
        for l, (hl, wl) in enumerate(GRIDS):
            ncell = hl * wl
            csx = float(wl) / (wl - 1.0)
            csy = float(hl) / (hl - 1.0)
            with tc.tile_pool(name="pc", bufs=1) as pc:
                dxp = pc.tile([128, 1024], F16, tag="dxp")
                dyp = pc.tile([128, 1024], F16, tag="dyp")
                # The whole coord pipeline runs in element order
                # u' = k*256 + up*128 + uw (dxp's DMA-natural layout).
                # Slot q' = u*16 + c with u = k*256 + uw*2 + up; sample
                # query qs = up*8192 + c*512 + h24*128 + uw.
                for mloc in range(2):
                    for xy, dst in ((0, dxp), (1, dyp)):
                        for k in range(4):
                            for h24 in range(4):
                                src = DLdv[mloc, l, k, xy, :].rearrange(
                                    "(up c h24 uw) -> h24 c up uw",
                                    up=2, c=16, h24=4, uw=128)[h24]
                                p0 = mloc * 64 + h24 * 16
                                nc.sync.dma_start(
                                    dst[p0:p0 + 16,
                                        k * 256:(k + 1) * 256]
                                    .rearrange("c (up uw) -> c up uw",
                                               up=2),
                                    src)
                phx = pc.tile([128, 1024], F32, tag="phx")
                nc.sync.dma_start(phx[:], pqx_d)
                phy = pc.tile([128, 1024], F32, tag="phy")
                nc.sync.dma_start(phy[:], pqy_d)
                if _DEBUG and l == 0:
                    nc.sync.dma_start(dbg["dbgdxp0"], dxp[:])
                    nc.sync.dma_start(dbg["dbgdyp0"], dyp[:])
                    nc.sync.dma_start(dbg["dbgphx"], phx[:])

                def coord(ph, dp, scale, cs, hi, itag, ctag):
                    t = pc.tile([128, 1024], F32, tag="s0")
                    nc.vector.tensor_scalar(t[:], ph[:], scale, None, ALU.mult)
                    nc.vector.tensor_tensor(t[:], t[:], dp[:], ALU.add)
                    ii = pc.tile([128, 1024], F32, tag=itag)
                    nc.scalar.activation(ii[:], t[:], AF.Copy, scale=cs,
                                         bias=-0.5)
                    # floor via round-nearest(v - 0.5) -> int32 -> f32
                    sh = pc.tile([128, 1024], F32, tag="s1")
                    nc.scalar.activation(sh[:], ii[:], AF.Copy, bias=-0.5)
                    iw = pc.tile([128, 1024], I32, tag="iw")
                    nc.vector.tensor_copy(iw[:], sh[:])
                    ff = pc.tile([128, 1024], F32, tag="s2s")
                    nc.vector.tensor_copy(ff[:], iw[:])
                    c0 = pc.tile([128, 1024], F32, tag="s3")
                    nc.vector.tensor_scalar_max(c0[:], ff[:], 0.0)
                    cc = pc.tile([128, 1024], F32, tag=ctag)
                    nc.vector.tensor_scalar_min(cc[:], c0[:], float(hi))
                    return ii, cc

                ix, cx = coord(phx, dxp, wl - 1.0, csx, wl - 2, "ix", "cx")
                iy, cy = coord(phy, dyp, hl - 1.0, csy, hl - 2, "iy", "cy")

                # idx_top = cy*(wl/2) + (cx>>1) + (cx&1)*(ncell/2)
                hf = pc.tile([128, 1024], F32, tag="s0")
                nc.scalar.activation(hf[:], cx[:], AF.Copy, scale=0.5,
                                     bias=-0.25)
                hfi = pc.tile([128, 1024], I32, tag="iw")
                nc.vector.tensor_copy(hfi[:], hf[:])
                nc.vector.tensor_copy(hf[:], hfi[:])
                par = pc.tile([128, 1024], F32, tag="s1")
                nc.vector.tensor_scalar(par[:], hf[:], -2.0, None, ALU.mult)
                nc.vector.tensor_tensor(par[:], par[:], cx[:], ALU.add)
                it = pc.tile([128, 1024], F32, tag="s2s")
                nc.scalar.activation(it[:], cy[:], AF.Copy, scale=wl / 2.0)
                nc.vector.tensor_tensor(it[:], it[:], hf[:], ALU.add)
                nc.vector.tensor_scalar(par[:], par[:], ncell / 2.0, None,
                                        ALU.mult)
                nc.vector.tensor_tensor(it[:], it[:], par[:], ALU.add)
                ib = pc.tile([128, 1024], F32, tag="s3")
                nc.vector.tensor_scalar(ib[:], it[:], wl / 2.0, None, ALU.add)
                # idxc window-major: col = wi*128 + up*64 + uwl*2 + tb
                # (wi = k*4 + uwq, uw = uwq*32 + uwl); it/ib are u'-ordered
                idv = idxc_t[l][:].rearrange(
                    "p (k uwq up uwl tb) -> p k tb uwq up uwl",
                    k=4, uwq=4, up=2, uwl=32, tb=2)
                for src_t, tb in ((it, 0), (ib, 1)):
                    sv = src_t[:].rearrange(
                        "p (k up uwq uwl) -> p k uwq up uwl",
                        k=4, up=2, uwq=4, uwl=32)
                    for k in range(4):
                        nc.vector.tensor_copy(idv[:, k, tb], sv[:, k])

                def tents(i_t, c_t, prtag):
                    tt = pc.tile([128, 1024], F32, tag="s0")
                    nc.vector.tensor_tensor(tt[:], i_t[:], c_t[:],
                                            ALU.subtract)
                    pair = pc.tile([128, 2048], F32, tag=prtag)
                    pv = pair[:].rearrange("p (n f) -> p n f", f=2)
                    ng = pc.tile([128, 1024], F32, tag="s1")
                    nc.vector.tensor_scalar(ng[:], tt[:], -1.0, None, ALU.mult)
                    a0 = pc.tile([128, 1024], F32, tag="s2s")
                    nc.vector.tensor_tensor(a0[:], tt[:], ng[:], ALU.max)
                    nc.scalar.activation(
                        pv[:, :, 0],
                        a0[:].rearrange("p (n o) -> p n o", o=1)[:, :, 0],
                        AF.Relu, scale=-1.0, bias=1.0)
                    a1 = pc.tile([128, 1024], F32, tag="s3")
                    nc.vector.tensor_scalar(a1[:], tt[:], -1.0, None, ALU.add)
                    ng1 = pc.tile([128, 1024], F32, tag="s1")
                    nc.vector.tensor_scalar(ng1[:], a1[:], -1.0, None,
                                            ALU.mult)
                    nc.vector.tensor_tensor(a1[:], a1[:], ng1[:], ALU.max)
                    nc.scalar.activation(
                        pv[:, :, 1],
                        a1[:].rearrange("p (n o) -> p n o", o=1)[:, :, 0],
                        AF.Relu, scale=-1.0, bias=1.0)
                    return pair

                u_pr = tents(ix, cx, "upr")
                v_pr = tents(iy, cy, "vpr")

                # W4c[p, (u, half, lr)] = v[half] * u[lr]
                w4v = W4c_t[l][:].rearrange("p (n a b) -> p n a b", a=2, b=2)
                nc.vector.tensor_tensor(
                    w4v,
                    v_pr[:].rearrange("p (n a o) -> p n a o", a=2, o=1)
                    .broadcast_to((P, 1024, 2, 2)),
                    u_pr[:].rearrange("p (n o b) -> p n o b", o=1, b=2)
                    .broadcast_to((P, 1024, 2, 2)),
                    ALU.mult)
        # ===== phase D: value projection -> img -> s2 pair planes =====
        for l, (hl, wl) in enumerate(GRIDS):
            ncell = hl * wl
            x_d = x0_d if l == 0 else x1_d
            with tc.tile_pool(name="pdi", bufs=1) as pdi, \
                 tc.tile_pool(name="pd", bufs=2) as pd_, \
                 tc.tile_pool(name="pdp", bufs=2, space="PSUM") as pdp, \
                 tc.tile_pool(name="pdp2", bufs=2, space="PSUM") as pdp2:
                img = pdi.tile([32, ncell + 8], BF16, tag="img")
                zpad = pd_.tile([32, 8], BF16, tag="zpad")
                nc.vector.memset(zpad[:], 0.0)
                nc.sync.dma_start(img[:, ncell:], zpad[:])
                for ch in range(ncell // 512):
                    xr = pd_.tile([128, 512], BF16, tag="xr")
                    nc.sync.dma_start(
                        xr[:].rearrange("p (j c) -> p j c", j=4),
                        x_d[ch * 512:(ch + 1) * 512, :]
                        .rearrange("(j p) c -> p j c", j=4))
                    xT = pd_.tile([128, 512], BF16, tag="xT")
                    for j in range(4):
                        tp = pdp.tile([128, 128], BF16, tag="xtp")
                        nc.tensor.transpose(
                            tp[:], xr[:, j * 128:(j + 1) * 128], eyeb_t[:])
                        nc.scalar.activation(xT[:, j * 128:(j + 1) * 128],
                                             tp[:], AF.Identity)
                    ips = pdp2.tile([32, 512], F32, tag="ips")
                    nc.tensor.matmul(ips[:], Wp2_t[:], xT[:], start=True,
                                     stop=True)
                    nc.scalar.activation(img[:, ch * 512:(ch + 1) * 512],
                                         ips[:], AF.Identity, bias=bp2_t[:])
                for mloc in range(2):
                    for kg in range(4):
                        g0 = mloc * 64 + kg * 16
                        rs = slice(mloc * 16, (mloc + 1) * 16)
                        nc.sync.dma_start(s2_t[l][g0:g0 + 16, :ncell],
                                          img[rs, :ncell])
                        nc.sync.dma_start(s2_t[l][g0:g0 + 16, ncell:],
                                          img[rs, 1:ncell + 1])

        # ===== phase E: gather, combine, project, transpose out =====
        with tc.tile_pool(name="pe", bufs=2) as pe, \
             tc.tile_pool(name="pe1", bufs=2) as pe1, \
             tc.tile_pool(name="pout", bufs=1, space="PSUM") as pout, \
             tc.tile_pool(name="pan", bufs=1, space="PSUM") as pan, \
             tc.tile_pool(name="ptp", bufs=2, space="PSUM") as ptp:
            for wi in range(NWIN):
                owin = pout.tile([128, 1024], F32, tag="owps")
                for l, (hl, wl) in enumerate(GRIDS):
                    ncell = hl * wl
                    anw = pe1.tile([16, 1024], BF16, tag="anw")
                    nc.sync.dma_start(anw[:],
                                      An_d[:, wi * 1024:(wi + 1) * 1024])
                    anp = pan.tile([128, 1024], F32, tag="anp")
                    for h in range(2):
                        nc.tensor.matmul(
                            anp[:, h * 512:(h + 1) * 512], Ra_t[:],
                            anw[:, h * 512:(h + 1) * 512],
                            start=True, stop=True)
                    g2 = pe.tile([128, 4096], BF16, tag="g2")
                    nc.gpsimd.ap_gather(
                        g2[:].rearrange("p (n d) -> p n d", d=2),
                        s2_t[l][:].rearrange("p (n d) -> p n d", d=2),
                        idxc_t[l][:, wi * 128:(wi + 1) * 128],
                        channels=128, num_elems=ncell, d=2, num_idxs=2048)
                    g2v = g2[:].rearrange("p (e tb c lr) -> p c e tb lr",
                                          e=64, tb=2, c=16, lr=2)
                    w4v = W4c_t[l][:].rearrange(
                        "p (k up uwq uwl hr) -> p k uwq up uwl hr",
                        k=4, up=2, uwq=4, uwl=32, hr=4)[:, wi // 4, wi % 4]
                    for cc in range(16):
                        wsh = pe1.tile([128, 256], BF16, tag="wsh")
                        nc.vector.stream_shuffle(
                            wsh[:].rearrange("p (up uwl hr) -> p up uwl hr",
                                             up=2, uwl=32),
                            w4v,
                            [(j // 16) * 16 + cc for j in range(32)])
                        gslice = g2v[:, cc:cc + 1, :, :, :].rearrange(
                            "p c e tb lr -> p (c e) tb lr")
                        nc.vector.tensor_tensor(
                            gslice, gslice,
                            wsh[:].rearrange("p (e h lr) -> p e h lr",
                                             e=64, h=2, lr=2),
                            ALU.mult)
                    r1 = pe1.tile([128, 2048], F32, tag="r1")
                    nc.vector.tensor_reduce(
                        r1[:], g2[:].rearrange("p (n lr) -> p n lr", lr=2),
                        axis=AX.X, op=ALU.add)
                    r1v = r1[:].rearrange(
                        "p (up uwl tb c) -> p tb up uwl c",
                        up=2, uwl=32, tb=2, c=16)
                    va = pe1.tile([128, 1024], F32, tag="va")
                    nc.vector.tensor_tensor(
                        va[:].rearrange("p (uwl up c) -> p up uwl c",
                                        uwl=32, up=2, c=16),
                        r1v[:, 0], r1v[:, 1], ALU.add)
                    v2 = pe1.tile([128, 1024], BF16, tag="v2")
                    nc.vector.tensor_tensor(v2[:], va[:], anp[:], ALU.mult)
                    for h in range(2):
                        nc.tensor.matmul(owin[:, h * 512:(h + 1) * 512],
                                         F_t[l][:],
                                         v2[:, h * 512:(h + 1) * 512],
                                         start=(l == 0), stop=(l == 1))
                ow = pe.tile([128, 1024], F32, tag="owsb")
                nc.scalar.activation(ow[:], owin[:], AF.Identity,
                                     bias=bmv_t[:])
                outw = pe.tile([128, 1024], F32, tag="outw")
                for j in range(8):
                    tp = ptp.tile([128, 128], F32, tag="otp")
                    nc.tensor.transpose(tp[:], ow[:, j * 128:(j + 1) * 128],
                                        eyef_t[:])
                    nc.vector.tensor_copy(outw[:, j * 128:(j + 1) * 128],
                                          tp[:])
                nc.sync.dma_start(
                    outT_d[wi * 1024:(wi + 1) * 1024, :]
                    .rearrange("(j p) c -> p j c", j=8),
                    outw[:].rearrange("p (j c) -> p j c", j=8))
            if _DEBUG:
                nc.sync.dma_start(dbg["dbgDLd"], DLd_d)
                nc.sync.dma_start(dbg["dbgAn"], An_d)
                nc.sync.dma_start(dbg["dbgidx0"], idxc_t[0][:])
                nc.sync.dma_start(dbg["dbgidx1"], idxc_t[1][:])
                nc.sync.dma_start(dbg["dbgw40"], W4c_t[0][:])
                nc.sync.dma_start(dbg["dbgw41"], W4c_t[1][:])
                nc.sync.dma_start(dbg["dbgoutT"], outT_d)
            nc.gpsimd.collective_compute(
                "ReduceScatter", ALU.add,
                replica_groups=[[0, 1, 2, 3], [4, 5, 6, 7]],
                ins=[outT_d], outs=[rs_d])
            nc.sync.dma_start(outp_d, rs_d)

    nc.compile()
    return nc


def _to_bf16(a):
    return np.asarray(a, np.float32).astype(ml_dtypes.bfloat16)


_PIDX = np.arange(128)
_MLOC = _PIDX // 64
_H24 = (_PIDX // 16) % 4
# element order u' = k*256 + up*128 + uw; sample query
# qs = ((up*16 + c)*4 + h24)*128 + uw
_UP = (np.arange(1024) // 128) % 2
_UW = np.arange(1024) % 128
_QS = (((_UP[None, :] * 16 + (_PIDX % 16)[:, None]) * 4
        + _H24[:, None]) * 128 + _UW[None, :])


def _host_prep(z_q, x0, x1, p_q, Wq, bq, Wd, bd, Wa, ba, Wp, bp, Wm, bm):
    f32 = np.float32
    Wqd_r = (Wq @ Wd).astype(f32).reshape(C, M, L, K, 2)
    bqd_r = (bq @ Wd + bd).astype(f32).reshape(M, L, K, 2)
    Wqa_r = (Wq @ Wa).astype(f32).reshape(C, M, L * K)
    bqa_r = (bq @ Wa + ba).astype(f32).reshape(M, L * K)
    Wp_r = Wp.reshape(C, M, C_v)
    bp_r = bp.reshape(M, C_v)

    # p_q gathered into the device (partition, slot) layout: partition
    # p = (mloc, h24, c), slot q' = u*16 + c, sample query
    # qs = ((u%2)*16 + c)*512 + h24*128 + (u//2)%128 -- implements the
    # reference's faithful scrambled permute/view pairing. phi uses
    # p_q[m % 2] = p_q[mloc] (faithful m*B+b vs b*M+m batch mismatch).
    pq = np.asarray(p_q, f32).reshape(2, HW, 2)
    pqx = pq[_MLOC[:, None], _QS, 0]
    pqy = pq[_MLOC[:, None], _QS, 1]

    zb = [_to_bf16(np.asarray(z_q[b]).reshape(HW, C)) for b in range(B)]
    x0b = [_to_bf16(np.asarray(x0[b]).reshape(-1, C)) for b in range(B)]
    x1b = [_to_bf16(np.asarray(x1[b]).reshape(-1, C)) for b in range(B)]

    maps = []
    for c in range(N_CORES):
        b = c // 4
        m0 = 2 * (c % 4)
        Wc = np.zeros((C, 48), f32)
        bcd = np.zeros((32, 1), f32)
        bca = np.zeros((16, 1), f32)
        for ml in range(2):
            m = m0 + ml
            Wc[:, ml * 16:(ml + 1) * 16] = Wqd_r[:, m].reshape(C, 16)
            bcd[ml * 16:(ml + 1) * 16, 0] = bqd_r[m].reshape(16)
            Wc[:, 32 + ml * 8:32 + (ml + 1) * 8] = Wqa_r[:, m]
            bca[ml * 8:(ml + 1) * 8, 0] = bqa_r[m]
        Wp2 = np.concatenate([Wp_r[:, m0], Wp_r[:, m0 + 1]], axis=1)
        bp2 = np.concatenate([bp_r[m0], bp_r[m0 + 1]])[:, None].astype(f32)
        Fs = []
        for l in range(2):
            rows = (m0 + _MLOC) * C_v + _H24 * 4 + l * 2 + (_PIDX % 16) // 8
            Fs.append(_to_bf16(Wm[rows].astype(f32)))
        lead = (c % 4) == 0
        maps.append(dict(
            zq=zb[b], x0=x0b[b], x1=x1b[b], pqx=pqx, pqy=pqy,
            Wcmb=_to_bf16(Wc), bcd=bcd, bca=bca,
            Wp2=_to_bf16(Wp2.astype(f32)), bp2=bp2, F0=Fs[0], F1=Fs[1],
            bmv=(np.asarray(bm, f32)[:, None].copy() if lead
                 else np.zeros((C, 1), f32)),
        ))
    return maps


def _install_err_capture():
    import traceback, subprocess
    from concourse import bass2jax as b2j
    if getattr(b2j, "_err_capture_installed", False):
        return
    orig = b2j.neuronx_cc_hook

    def wrapped(*a, **k):
        try:
            return orig(*a, **k)
        except BaseException as e:
            with open("/tmp/ncc_hook_err.txt", "w") as f:
                f.write(traceback.format_exc())
                ee = e
                while ee is not None:
                    if isinstance(ee, subprocess.CalledProcessError):
                        so = ee.stdout if isinstance(ee.stdout, str) else (
                            ee.stdout or b"").decode(errors="replace")
                        f.write("\n==== STDOUT-tail ====\n" + so[-4000:])
                    ee = ee.__cause__ or ee.__context__
            raise

    b2j.neuronx_cc_hook = wrapped
    b2j._err_capture_installed = True
    import libneuronxla
    libneuronxla.neuronx_cc = wrapped


class CachedRunner:
    """Build the shard_map jit wrapper for a Bass program once and reuse it
    for every call (run_bass_kernel_spmd rebuilds and retraces per call)."""

    def __init__(self, nc, n_cores=N_CORES):
        import jax
        from jax.sharding import Mesh, PartitionSpec
        from jax.experimental.shard_map import shard_map
        from concourse.bass2jax import (
            _bass_exec_p, partition_id_tensor, install_neuronx_cc_hook)
        install_neuronx_cc_hook()
        self.nc = nc
        self.n_cores = n_cores
        partition_name = (nc.partition_id_tensor.name
                          if nc.partition_id_tensor else None)
        in_names, out_names, out_avals, zero_shapes = [], [], [], []
        for alloc in nc.m.functions[0].allocations:
            if not isinstance(alloc, mybir.MemoryLocationSet):
                continue
            name = alloc.memorylocations[0].name
            if alloc.kind == "ExternalInput":
                if name != partition_name:
                    in_names.append(name)
            elif alloc.kind == "ExternalOutput":
                shape = tuple(alloc.tensor_shape)
                dtype = mybir.dt.np(alloc.dtype)
                out_avals.append(jax.core.ShapedArray(shape, dtype))
                out_names.append(name)
                zero_shapes.append((shape, dtype))
        self.in_names = list(in_names)
        self.out_names = out_names
        self.out_avals = out_avals
        self.zero_shapes = zero_shapes
        n_params = len(in_names)
        n_outs = len(out_avals)
        all_names = list(in_names) + list(out_names)
        if partition_name is not None:
            all_names.append(partition_name)
        donate = tuple(range(n_params, n_params + n_outs))

        def _body(*args):
            operands = list(args)
            if partition_name is not None:
                operands.append(partition_id_tensor())
            outs = _bass_exec_p.bind(
                *operands,
                out_avals=tuple(out_avals),
                in_names=tuple(all_names),
                out_names=tuple(out_names),
                lowering_input_output_aliases=(),
                sim_require_finite=True,
                sim_require_nnan=True,
                nc=nc,
            )
            return tuple(outs)

        devices = jax.devices()[:n_cores]
        mesh = Mesh(np.asarray(devices), ("core",))
        in_specs = (PartitionSpec("core"),) * (n_params + n_outs)
        out_specs = (PartitionSpec("core"),) * n_outs
        self._fn = jax.jit(
            shard_map(_body, mesh=mesh, in_specs=in_specs,
                      out_specs=out_specs, check_rep=False),
            donate_argnums=donate, keep_unused=True)

    def __call__(self, concat_inputs):
        """concat_inputs: arrays of shape (n_cores*dim0, ...) in in_names
        order. Returns list of np arrays (n_cores, *out_shape)."""
        zeros = [np.zeros((self.n_cores * s[0], *s[1:]), d)
                 for s, d in self.zero_shapes]
        outs = self._fn(*concat_inputs, *zeros)
        return [np.asarray(o).reshape(self.n_cores, *self.out_avals[i].shape)
                for i, o in enumerate(outs)]


def _concat_from_maps(runner, maps):
    return [np.concatenate([np.asarray(m[name]) for m in maps], axis=0)
            for name in runner.in_names]


def _fill_concat(runner, inputs):
    """Fill preallocated per-input concat buffers directly (avoids the
    intermediate per-core maps + np.concatenate copies)."""
    z_q, x0, x1, p_q = (inputs["z_q"], inputs["x0"], inputs["x1"],
                        inputs["p_q"])
    bufs = _CACHED.get("bufs")
    if bufs is None:
        bufs = {}
        shapes = dict(zq=(HW, C), x0=(4096, C), x1=(HW, C),
                      pqx=(128, 1024), pqy=(128, 1024))
        dts = dict(zq=ml_dtypes.bfloat16, x0=ml_dtypes.bfloat16,
                   x1=ml_dtypes.bfloat16, pqx=np.float32, pqy=np.float32)
        for nm, shp in shapes.items():
            bufs[nm] = np.empty((N_CORES * shp[0], *shp[1:]), dts[nm])
        _CACHED["bufs"] = bufs

    for nm, full in (("zq", z_q), ("x0", x0), ("x1", x1)):
        v = bufs[nm].reshape(N_CORES, -1, C)
        for b in range(B):
            np.copyto(v[b * 4], np.asarray(full[b]).reshape(-1, C),
                      casting="unsafe")
            for g in range(1, 4):
                v[b * 4 + g] = v[b * 4]
    pq = np.asarray(p_q, np.float32).reshape(2, HW, 2)
    pqx = pq[_MLOC[:, None], _QS, 0]
    pqy = pq[_MLOC[:, None], _QS, 1]
    vx = bufs["pqx"].reshape(N_CORES, 128, 1024)
    vy = bufs["pqy"].reshape(N_CORES, 128, 1024)
    for c in range(N_CORES):
        vx[c] = pqx
        vy[c] = pqy

    # small per-core weight tensors via the regular path
    small = _host_prep_small(**{k: np.asarray(v) for k, v in inputs.items()})
    out = []
    for name in runner.in_names:
        if name in bufs:
            out.append(bufs[name])
        else:
            out.append(np.concatenate(
                [np.asarray(m[name]) for m in small], axis=0))
    return out


def _host_prep_small(z_q, x0, x1, p_q, Wq, bq, Wd, bd, Wa, ba, Wp, bp,
                     Wm, bm):
    f32 = np.float32
    Wqd_r = (Wq @ Wd).astype(f32).reshape(C, M, L, K, 2)
    bqd_r = (bq @ Wd + bd).astype(f32).reshape(M, L, K, 2)
    Wqa_r = (Wq @ Wa).astype(f32).reshape(C, M, L * K)
    bqa_r = (bq @ Wa + ba).astype(f32).reshape(M, L * K)
    Wp_r = Wp.reshape(C, M, C_v)
    bp_r = bp.reshape(M, C_v)
    maps = []
    for c in range(N_CORES):
        m0 = 2 * (c % 4)
        Wc = np.zeros((C, 48), f32)
        bcd = np.zeros((32, 1), f32)
        bca = np.zeros((16, 1), f32)
        for ml in range(2):
            m = m0 + ml
            Wc[:, ml * 16:(ml + 1) * 16] = Wqd_r[:, m].reshape(C, 16)
            bcd[ml * 16:(ml + 1) * 16, 0] = bqd_r[m].reshape(16)
            Wc[:, 32 + ml * 8:32 + (ml + 1) * 8] = Wqa_r[:, m]
            bca[ml * 8:(ml + 1) * 8, 0] = bqa_r[m]
        Wp2 = np.concatenate([Wp_r[:, m0], Wp_r[:, m0 + 1]], axis=1)
        bp2 = np.concatenate([bp_r[m0], bp_r[m0 + 1]])[:, None].astype(f32)
        Fs = []
        for l in range(2):
            rows = (m0 + _MLOC) * C_v + _H24 * 4 + l * 2 + (_PIDX % 16) // 8
            Fs.append(_to_bf16(Wm[rows].astype(f32)))
        lead = (c % 4) == 0
        maps.append(dict(
            Wcmb=_to_bf16(Wc), bcd=bcd, bca=bca,
            Wp2=_to_bf16(Wp2.astype(f32)), bp2=bp2, F0=Fs[0], F1=Fs[1],
            bmv=(np.asarray(bm, f32)[:, None].copy() if lead
                 else np.zeros((C, 1), f32)),
        ))
    return maps


def kernel(**inputs):
    _install_err_capture()
    if "runner" not in _CACHED:
        _CACHED["nc"] = _build_program()
        _CACHED["runner"] = CachedRunner(_CACHED["nc"])
    runner = _CACHED["runner"]
    res = runner(_fill_concat(runner, inputs))[0]  # [8, 4096, C]
    # shard order (b, g, q, c) is exactly the output layout
    return np.ascontiguousarray(res).reshape(B, H, W, C)
